# revision 2
# baseline (speedup 1.0000x reference)
"""LGnet (LSTM-style gated recurrent net) Trainium2 Bass kernel.

B=64, T=256, D=512, H=512, COMB=1536. Data-parallel over batch across 8
NeuronCores (B_local=8 per core).

Decomposition: comb @ W = xt' @ W[0:512] + h' @ W[512:1024] + m @ W[1024:1536].
Only the h' term is recurrent; everything else (xt', delta_x, delta_h, the
xt'/m gate contributions "a") is precomputed for all (t, b) with large
matmuls.  The sequential phase then does one [8,512] @ [512,2048] bf16
matmul per step in weight-stationary transposed-gate layout so the
elementwise chain runs on full 128-partition tiles.

Layouts (per core), R = T*B_l = 2048, row index r = t*8 + b:
  xtT/xlT/mT/dlT : [4, 128, R]   field^T chunks   [d_chunk, d_in_chunk, r]
  xmT            : [4, 128, R]   X_mean^T broadcast over b
  Wg             : [8, 128, 2048] non-recurrent gate weights (K= xt rows 0:512
                   then m rows 1024:1536), N = [i|f|o|c] * 512
  Wh             : [4, 128, 2048] bf16, recurrent rows 512:1024
  ghW            : [4, 128, 512]
  pvec           : [128, 28] per-partition params: gxw(4) gxb(4) ghb(4) bg(16)
  hT_out         : [4, 128, 256, 8]  h^T staged output [hc, p, t, b]
"""

import os
import numpy as np
import ml_dtypes

B, T, D, H = 64, 256, 512, 512
NCORES = 8
BL = B // NCORES          # 8 per-core batch
R = T * BL                # 2048 rows
DC, HC, GC = 4, 4, 16     # 128-chunks of D, H, and 4*H gate cols
G = 4 * H                 # 2048 gate columns
SS = 16                   # recurrence steps per staging block

_CACHE = {}


def _build():
    import concourse.bacc as bacc
    import concourse.bass as bass
    import concourse.tile as tile
    import concourse.mybir as mybir

    f32 = mybir.dt.float32
    bf16 = mybir.dt.bfloat16
    AF = mybir.ActivationFunctionType

    nc = bacc.Bacc("TRN2", target_bir_lowering=False, debug=False)

    dt_in = {}
    for name, shape in [
        ("xtT", (DC, 128, R)), ("xlT", (DC, 128, R)), ("mTd", (DC, 128, R)),
        ("dlT", (DC, 128, R)), ("xmT", (DC, 128, R)),
        ("Wg", (8, 128, G)), ("ghW", (DC, 128, D)), ("pvec", (128, 28)),
    ]:
        dt_in[name] = nc.dram_tensor(name, list(shape), f32, kind="ExternalInput").ap()
    dt_in["Wh"] = nc.dram_tensor("Wh", [HC, 128, G], bf16, kind="ExternalInput").ap()
    out_ap = nc.dram_tensor("hT_out", [HC, 128, T, BL], f32, kind="ExternalOutput").ap()

    with tile.TileContext(nc) as tc:
        with (
            tc.tile_pool(name="const", bufs=1) as cp,
            tc.tile_pool(name="work", bufs=2) as wp,
            tc.tile_pool(name="wg", bufs=2) as wgp,
            tc.tile_pool(name="ablk", bufs=2) as abp,
            tc.tile_pool(name="stage", bufs=2) as stp,
            tc.tile_pool(name="psum", bufs=4, space="PSUM") as pp,
            tc.tile_pool(name="psum3", bufs=2, space="PSUM") as pp3,
            tc.tile_pool(name="dram", bufs=1, space="DRAM") as dp,
        ):
            # ---- resident tiles ----
            pvec = cp.tile([128, 28], f32)
            nc.sync.dma_start(pvec[:], dt_in["pvec"][:])
            ghw = cp.tile([128, DC, D], f32)
            for kc in range(DC):
                nc.sync.dma_start(ghw[:, kc, :], dt_in["ghW"][kc])
            whs = cp.tile([128, HC, G], bf16)
            for kc in range(HC):
                nc.sync.dma_start(whs[:, kc, :], dt_in["Wh"][kc])
            xtp = cp.tile([128, DC, R], f32)      # xt'
            mres = cp.tile([128, DC, R], f32)     # m^T resident
            dht = cp.tile([128, HC, R], f32)      # delta_h^T resident
            aT = dp.tile([GC, 128, R], f32)       # gate preactivation staging

            NB = R // 512  # 4 blocks of 512 rows

            # ---- phase 1b: delta_h = exp(-relu(dl @ ghW + ghb)) ----
            for nb in range(NB):
                dl4 = wp.tile([128, DC, 512], f32, tag="dl4")
                for kc in range(DC):
                    nc.sync.dma_start(
                        dl4[:, kc, :], dt_in["dlT"][kc][:, nb * 512:(nb + 1) * 512])
                for mt in range(HC):
                    ps = pp.tile([128, 512], f32, tag="mmps")
                    for kc in range(DC):
                        nc.tensor.matmul(
                            ps[:], ghw[:, kc, mt * 128:(mt + 1) * 128],
                            dl4[:, kc, :], start=(kc == 0), stop=(kc == DC - 1))
                    t1 = wp.tile([128, 512], f32, tag="dht1")
                    nc.scalar.activation(t1[:], ps[:], AF.Relu,
                                         bias=pvec[:, 8 + mt:9 + mt], scale=1.0)
                    nc.scalar.activation(
                        dht[:, mt, nb * 512:(nb + 1) * 512], t1[:], AF.Exp,
                        scale=-1.0)

            # ---- phase 1: xt' (elementwise) ----
            for d in range(DC):
                for nb in range(NB):
                    sl = slice(nb * 512, (nb + 1) * 512)
                    xt = wp.tile([128, 512], f32, tag="xt")
                    xl = wp.tile([128, 512], f32, tag="xl")
                    dl = wp.tile([128, 512], f32, tag="dl")
                    xm = wp.tile([128, 512], f32, tag="xm")
                    nc.sync.dma_start(xt[:], dt_in["xtT"][d][:, sl])
                    nc.sync.dma_start(xl[:], dt_in["xlT"][d][:, sl])
                    nc.sync.dma_start(dl[:], dt_in["dlT"][d][:, sl])
                    nc.sync.dma_start(xm[:], dt_in["xmT"][d][:, sl])
                    nc.sync.dma_start(mres[:, d, sl], dt_in["mTd"][d][:, sl])
                    dx = wp.tile([128, 512], f32, tag="dx")
                    nc.scalar.activation(dx[:], dl[:], AF.Relu,
                                         bias=pvec[:, 4 + d:5 + d],
                                         scale=pvec[:, d:d + 1])
                    nc.scalar.activation(dx[:], dx[:], AF.Exp, scale=-1.0)
                    # inner = dx*(xl-xm) + xm ; xt' = m*(xt-inner) + inner
                    s1 = wp.tile([128, 512], f32, tag="s1")
                    nc.vector.tensor_sub(s1[:], xl[:], xm[:])
                    nc.vector.tensor_mul(s1[:], dx[:], s1[:])
                    nc.vector.tensor_add(s1[:], s1[:], xm[:])      # inner
                    s2 = wp.tile([128, 512], f32, tag="s2")
                    nc.vector.tensor_sub(s2[:], xt[:], s1[:])
                    nc.vector.tensor_mul(s2[:], mres[:, d, sl], s2[:])
                    nc.vector.tensor_add(xtp[:, d, sl], s2[:], s1[:])

            # ---- phase 2: a = xt'@Wx + m@Wm + bias  -> aT dram ----
            for gc in range(GC):
                wg = wgp.tile([128, 8, 128], f32, tag="wg")
                for kc in range(8):
                    nc.sync.dma_start(
                        wg[:, kc, :], dt_in["Wg"][kc][:, gc * 128:(gc + 1) * 128])
                for nb in range(NB):
                    sl = slice(nb * 512, (nb + 1) * 512)
                    ps = pp.tile([128, 512], f32, tag="mmps")
                    for kc in range(8):
                        rhs = xtp[:, kc, sl] if kc < DC else mres[:, kc - DC, sl]
                        nc.tensor.matmul(ps[:], wg[:, kc, :], rhs,
                                         start=(kc == 0), stop=(kc == 7))
                    ao = wp.tile([128, 512], f32, tag="ao")
                    nc.scalar.activation(ao[:], ps[:], AF.Identity,
                                         bias=pvec[:, 12 + gc:13 + gc], scale=1.0)
                    nc.sync.dma_start(aT[gc][:, sl], ao[:])

            # ---- phase 3: recurrence ----
            c_st = cp.tile([128, HC, BL], f32)
            hbf = cp.tile([128, HC, BL], bf16)
            nc.vector.memset(c_st[:], 0.0)
            nc.vector.memset(hbf[:], 0.0)

            for blk in range(T // SS):
                t0 = blk * SS
                ab = abp.tile([128, GC, SS * BL], f32, tag="ab")
                for gc in range(GC):
                    nc.sync.dma_start(
                        ab[:, gc, :], aT[gc][:, t0 * BL:(t0 + SS) * BL])
                hst = stp.tile([128, SS, HC, BL], f32, tag="hst")
                for s in range(SS):
                    t = t0 + s
                    gps = pp3.tile([128, GC, BL], f32, tag="gps")
                    for gc in range(GC):
                        for kc in range(HC):
                            nc.tensor.matmul(
                                gps[:, gc, :],
                                whs[:, kc, gc * 128:(gc + 1) * 128],
                                hbf[:, kc, :],
                                start=(kc == 0), stop=(kc == HC - 1))
                    g = wp.tile([128, GC, BL], f32, tag="g")
                    nc.vector.tensor_add(g[:], gps[:],
                                         ab[:, :, s * BL:(s + 1) * BL])
                    ga = wp.tile([128, GC, BL], f32, tag="ga")
                    nc.scalar.activation(ga[:, 0:12, :], g[:, 0:12, :], AF.Sigmoid)
                    nc.scalar.activation(ga[:, 12:16, :], g[:, 12:16, :], AF.Tanh)
                    tn = min(t + 1, T - 1)
                    odh = wp.tile([128, HC, BL], f32, tag="odh")
                    nc.vector.tensor_mul(odh[:], ga[:, 8:12, :],
                                         dht[:, :, tn * BL:(tn + 1) * BL])
                    tmp = wp.tile([128, HC, BL], f32, tag="tmp")
                    nc.vector.tensor_mul(tmp[:], ga[:, 0:4, :], ga[:, 12:16, :])
                    nc.vector.tensor_mul(c_st[:], c_st[:], ga[:, 4:8, :])
                    nc.vector.tensor_add(c_st[:], c_st[:], tmp[:])
                    th = wp.tile([128, HC, BL], f32, tag="th")
                    nc.scalar.activation(th[:], c_st[:], AF.Tanh)
                    nc.vector.tensor_mul(hst[:, s, :, :], ga[:, 8:12, :], th[:])
                    nc.vector.tensor_mul(hbf[:], odh[:], th[:])
                for hc in range(HC):
                    nc.sync.dma_start(out_ap[hc][:, t0:t0 + SS, :],
                                      hst[:, :, hc, :])

    nc.compile()
    return nc


def _prep_inputs(x, X_mean, Wi, bi, Wf, bf, Wo, bo, Wc, bc,
                 gx_w, gx_b, gh_W, gh_b):
    f32 = np.float32
    Wfull = np.concatenate([Wi, Wf, Wo, Wc], axis=1).astype(f32)   # [1536, 2048]
    bfull = np.concatenate([bi, bf, bo, bc]).astype(f32)           # [2048]
    Wg = np.concatenate([Wfull[0:512], Wfull[1024:1536]], axis=0
                        ).reshape(8, 128, G).copy()
    Wh = Wfull[512:1024].reshape(HC, 128, G).astype(ml_dtypes.bfloat16)
    ghW = gh_W.astype(f32).reshape(DC, 128, D).copy()
    pvec = np.zeros((128, 28), f32)
    pvec[:, 0:4] = gx_w.astype(f32).reshape(4, 128).T
    pvec[:, 4:8] = gx_b.astype(f32).reshape(4, 128).T
    pvec[:, 8:12] = gh_b.astype(f32).reshape(4, 128).T
    pvec[:, 12:28] = bfull.reshape(16, 128).T
    xmT = np.ascontiguousarray(
        np.repeat(X_mean.astype(f32).T[:, :, None], BL, axis=2)
        .reshape(DC, 128, R))

    shared = {"Wg": Wg, "Wh": Wh, "ghW": ghW, "pvec": pvec, "xmT": xmT}
    in_maps = []
    for c in range(NCORES):
        m = dict(shared)
        for fi, name in enumerate(["xtT", "xlT", "mTd", "dlT"]):
            arr = x[c * BL:(c + 1) * BL, fi]          # [8, 256, 512]
            m[name] = np.ascontiguousarray(
                arr.transpose(2, 1, 0).reshape(DC, 128, R)).astype(f32)
        in_maps.append(m)
    return in_maps


def kernel(**inputs):
    from concourse.bass_utils import run_bass_kernel_spmd

    if "nc" not in _CACHE:
        _CACHE["nc"] = _build()
    nc = _CACHE["nc"]

    in_maps = _prep_inputs(**inputs)
    res = run_bass_kernel_spmd(
        nc, in_maps, core_ids=list(range(NCORES)),
        trace=bool(int(os.environ.get("LG_TRACE", "0"))))
    _CACHE["last_result"] = res

    outs = []
    for c in range(NCORES):
        hT = res.results[c]["hT_out"]                 # [4, 128, 256, 8]
        outs.append(np.ascontiguousarray(
            hT.transpose(3, 2, 0, 1).reshape(BL, T, H)))
    return np.concatenate(outs, axis=0).astype(np.float32)


# revision 6
# speedup vs baseline: 1.6260x; 1.6260x over previous
"""LGnet (LSTM-style gated recurrent net) Trainium2 Bass kernel.

B=64, T=256, D=512, H=512, COMB=1536. Data-parallel over batch across 8
NeuronCores (B_local=8 per core).

Decomposition: comb @ W = xt' @ W[0:512] + h' @ W[512:1024] + m @ W[1024:1536].
Only the h' term is recurrent; everything else (xt', delta_x, delta_h, the
xt'/m gate contributions "a") is precomputed for all (t, b) with large
matmuls.  The sequential phase then does one [8,512] @ [512,2048] bf16
matmul per step in weight-stationary transposed-gate layout so the
elementwise chain runs on full 128-partition tiles.

Layouts (per core), R = T*B_l = 2048, row index r = t*8 + b:
  xtT/xlT/mT/dlT : [4, 128, R]   field^T chunks   [d_chunk, d_in_chunk, r]
  xmT            : [4, 128, R]   X_mean^T broadcast over b
  Wg             : [8, 128, 2048] non-recurrent gate weights (K= xt rows 0:512
                   then m rows 1024:1536), N = [i|f|o|c] * 512
  Wh             : [4, 128, 2048] bf16, recurrent rows 512:1024
  ghW            : [4, 128, 512]
  pvec           : [128, 28] per-partition params: gxw(4) gxb(4) ghb(4) bg(16)
  hT_out         : [4, 128, 256, 8]  h^T staged output [hc, p, t, b]
"""

import os
import numpy as np
import ml_dtypes

B, T, D, H = 64, 256, 512, 512
NCORES = 8
BL = B // NCORES          # 8 per-core batch
R = T * BL                # 2048 rows
DC, HC, GC = 4, 4, 16     # 128-chunks of D, H, and 4*H gate cols
G = 4 * H                 # 2048 gate columns
SS = 16                   # recurrence steps per staging block

_CACHE = {}


def _build():
    import concourse.bacc as bacc
    import concourse.bass as bass
    import concourse.tile as tile
    import concourse.mybir as mybir

    f32 = mybir.dt.float32
    f32r = mybir.dt.float32r
    bf16 = mybir.dt.bfloat16
    AF = mybir.ActivationFunctionType

    nc = bacc.Bacc("TRN2", target_bir_lowering=False, debug=False)

    dt_in = {}
    for name, shape in [
        ("xtT", (DC, 128, R)), ("xlT", (DC, 128, R)), ("mTd", (DC, 128, R)),
        ("dlT", (DC, 128, R)), ("xmT", (DC, 128, R)),
        ("Wg", (8, 128, G)), ("ghW", (DC, 128, D)), ("pvec", (128, 28)),
    ]:
        dt_in[name] = nc.dram_tensor(name, list(shape), f32, kind="ExternalInput").ap()
    dt_in["Wh"] = nc.dram_tensor("Wh", [HC, 128, G], bf16, kind="ExternalInput").ap()
    out_ap = nc.dram_tensor("hT_out", [HC, 128, T, BL], f32, kind="ExternalOutput").ap()

    with tile.TileContext(nc) as tc:
        with (
            tc.tile_pool(name="const", bufs=1) as cp,
            tc.tile_pool(name="work", bufs=2) as wp,
            tc.tile_pool(name="wg", bufs=2) as wgp,
            tc.tile_pool(name="ablk", bufs=2) as abp,
            tc.tile_pool(name="stage", bufs=2) as stp,
            tc.tile_pool(name="psum", bufs=4, space="PSUM") as pp,
            tc.tile_pool(name="psum3", bufs=2, space="PSUM") as pp3,
            tc.tile_pool(name="dram", bufs=1, space="DRAM") as dp,
        ):
            # ---- resident tiles ----
            pvec = cp.tile([128, 28], f32)
            nc.sync.dma_start(pvec[:], dt_in["pvec"][:])
            ghw = cp.tile([128, DC, D], f32r)
            for kc in range(DC):
                nc.sync.dma_start(ghw[:, kc, :], dt_in["ghW"][kc].bitcast(f32r))
            whs = cp.tile([128, HC, G], bf16)
            for kc in range(HC):
                nc.sync.dma_start(whs[:, kc, :], dt_in["Wh"][kc])
            xtp = cp.tile([128, DC, R], f32r)     # xt'
            mres = cp.tile([128, DC, R], f32r)    # m^T resident
            dht = cp.tile([128, HC, R], f32)      # delta_h^T resident
            aT = dp.tile([GC, 128, R], f32)       # gate preactivation staging

            NB = R // 512  # 4 blocks of 512 rows

            # ---- phase 1b: delta_h = exp(-relu(dl @ ghW + ghb)) ----
            for nb in range(NB):
                dl4 = wp.tile([128, DC, 512], f32r, tag="dl4")
                for kc in range(DC):
                    nc.sync.dma_start(
                        dl4[:, kc, :],
                        dt_in["dlT"][kc][:, nb * 512:(nb + 1) * 512].bitcast(f32r))
                for mt in range(HC):
                    ps = pp.tile([128, 512], f32, tag="mmps")
                    for kc in range(DC):
                        nc.tensor.matmul(
                            ps[:],
                            ghw[:, kc, mt * 128:(mt + 1) * 128],
                            dl4[:, kc, :],
                            start=(kc == 0), stop=(kc == DC - 1))
                    t1 = wp.tile([128, 512], f32, tag="dht1")
                    nc.scalar.activation(t1[:], ps[:], AF.Relu,
                                         bias=pvec[:, 8 + mt:9 + mt], scale=1.0)
                    nc.scalar.activation(
                        dht[:, mt, nb * 512:(nb + 1) * 512], t1[:], AF.Exp,
                        scale=-1.0)

            # ---- phase 1: xt' (elementwise) ----
            for d in range(DC):
                for nb in range(NB):
                    sl = slice(nb * 512, (nb + 1) * 512)
                    xt = wp.tile([128, 512], f32, tag="xt")
                    xl = wp.tile([128, 512], f32, tag="xl")
                    dl = wp.tile([128, 512], f32, tag="dl")
                    xm = wp.tile([128, 512], f32, tag="xm")
                    nc.sync.dma_start(xt[:], dt_in["xtT"][d][:, sl])
                    nc.sync.dma_start(xl[:], dt_in["xlT"][d][:, sl])
                    nc.sync.dma_start(dl[:], dt_in["dlT"][d][:, sl])
                    nc.sync.dma_start(xm[:], dt_in["xmT"][d][:, sl])
                    nc.sync.dma_start(mres[:, d, sl], dt_in["mTd"][d][:, sl].bitcast(f32r))
                    dx = wp.tile([128, 512], f32, tag="dx")
                    nc.scalar.activation(dx[:], dl[:], AF.Relu,
                                         bias=pvec[:, 4 + d:5 + d],
                                         scale=pvec[:, d:d + 1])
                    nc.scalar.activation(dx[:], dx[:], AF.Exp, scale=-1.0)
                    # inner = dx*(xl-xm) + xm ; xt' = m*(xt-inner) + inner
                    s1 = wp.tile([128, 512], f32, tag="s1")
                    nc.vector.tensor_sub(s1[:], xl[:], xm[:])
                    nc.vector.tensor_mul(s1[:], dx[:], s1[:])
                    nc.vector.tensor_add(s1[:], s1[:], xm[:])      # inner
                    s2 = wp.tile([128, 512], f32, tag="s2")
                    nc.vector.tensor_sub(s2[:], xt[:], s1[:])
                    nc.vector.tensor_mul(s2[:], mres[:, d, sl].bitcast(f32), s2[:])
                    nc.vector.tensor_add(xtp[:, d, sl], s2[:], s1[:])

            # ---- phase 2: a = xt'@Wx + m@Wm + bias  -> aT dram ----
            for gc in range(GC):
                wg = wgp.tile([128, 8, 128], f32r, tag="wg")
                for kc in range(8):
                    nc.sync.dma_start(
                        wg[:, kc, :],
                        dt_in["Wg"][kc][:, gc * 128:(gc + 1) * 128].bitcast(f32r))
                for nb in range(NB):
                    sl = slice(nb * 512, (nb + 1) * 512)
                    ps = pp.tile([128, 512], f32, tag="mmps")
                    for kc in range(8):
                        rhs = xtp[:, kc, sl] if kc < DC else mres[:, kc - DC, sl]
                        nc.tensor.matmul(ps[:], wg[:, kc, :],
                                         rhs,
                                         start=(kc == 0), stop=(kc == 7))
                    ao = wp.tile([128, 512], f32, tag="ao")
                    nc.scalar.activation(ao[:], ps[:], AF.Identity,
                                         bias=pvec[:, 12 + gc:13 + gc], scale=1.0)
                    nc.sync.dma_start(aT[gc][:, sl], ao[:])

            # ---- phase 3: recurrence ----
            c_st = cp.tile([128, HC, BL], f32)
            hbf = cp.tile([128, HC, BL], bf16)
            nc.vector.memset(c_st[:], 0.0)
            nc.vector.memset(hbf[:], 0.0)

            for blk in range(T // SS):
                t0 = blk * SS
                ab = abp.tile([128, GC, SS * BL], f32, tag="ab")
                for gc in range(GC):
                    nc.sync.dma_start(
                        ab[:, gc, :], aT[gc][:, t0 * BL:(t0 + SS) * BL])
                hst = stp.tile([128, SS, HC, BL], f32, tag="hst")
                for s in range(SS):
                    t = t0 + s
                    gps = pp3.tile([128, GC, BL], f32, tag="gps")
                    for gc in range(GC):
                        for kc in range(HC):
                            nc.tensor.matmul(
                                gps[:, gc, :],
                                whs[:, kc, gc * 128:(gc + 1) * 128],
                                hbf[:, kc, :],
                                start=(kc == 0), stop=(kc == HC - 1))
                    g = wp.tile([128, GC, BL], f32, tag="g")
                    nc.vector.tensor_add(g[:], gps[:],
                                         ab[:, :, s * BL:(s + 1) * BL])
                    ga = wp.tile([128, GC, BL], f32, tag="ga")
                    nc.scalar.activation(ga[:, 0:12, :], g[:, 0:12, :], AF.Sigmoid)
                    nc.scalar.activation(ga[:, 12:16, :], g[:, 12:16, :], AF.Tanh)
                    tn = min(t + 1, T - 1)
                    odh = wp.tile([128, HC, BL], f32, tag="odh")
                    nc.vector.tensor_mul(odh[:], ga[:, 8:12, :],
                                         dht[:, :, tn * BL:(tn + 1) * BL])
                    tmp = wp.tile([128, HC, BL], f32, tag="tmp")
                    nc.vector.tensor_mul(tmp[:], ga[:, 0:4, :], ga[:, 12:16, :])
                    nc.vector.tensor_mul(c_st[:], c_st[:], ga[:, 4:8, :])
                    nc.vector.tensor_add(c_st[:], c_st[:], tmp[:])
                    th = wp.tile([128, HC, BL], f32, tag="th")
                    nc.scalar.activation(th[:], c_st[:], AF.Tanh)
                    nc.vector.tensor_mul(hst[:, s, :, :], ga[:, 8:12, :], th[:])
                    nc.vector.tensor_mul(hbf[:], odh[:], th[:])
                for hc in range(HC):
                    nc.sync.dma_start(out_ap[hc][:, t0:t0 + SS, :],
                                      hst[:, :, hc, :])

    nc.compile()
    return nc


def _prep_inputs(x, X_mean, Wi, bi, Wf, bf, Wo, bo, Wc, bc,
                 gx_w, gx_b, gh_W, gh_b):
    f32 = np.float32
    Wfull = np.concatenate([Wi, Wf, Wo, Wc], axis=1).astype(f32)   # [1536, 2048]
    bfull = np.concatenate([bi, bf, bo, bc]).astype(f32)           # [2048]
    Wg = np.concatenate([Wfull[0:512], Wfull[1024:1536]], axis=0
                        ).reshape(8, 128, G).copy()
    Wh = Wfull[512:1024].reshape(HC, 128, G).astype(ml_dtypes.bfloat16)
    ghW = gh_W.astype(f32).reshape(DC, 128, D).copy()
    pvec = np.zeros((128, 28), f32)
    pvec[:, 0:4] = gx_w.astype(f32).reshape(4, 128).T
    pvec[:, 4:8] = gx_b.astype(f32).reshape(4, 128).T
    pvec[:, 8:12] = gh_b.astype(f32).reshape(4, 128).T
    pvec[:, 12:28] = bfull.reshape(16, 128).T
    xmT = np.ascontiguousarray(
        np.repeat(X_mean.astype(f32).T[:, :, None], BL, axis=2)
        .reshape(DC, 128, R))

    shared = {"Wg": Wg, "Wh": Wh, "ghW": ghW, "pvec": pvec, "xmT": xmT}
    in_maps = []
    for c in range(NCORES):
        m = dict(shared)
        for fi, name in enumerate(["xtT", "xlT", "mTd", "dlT"]):
            arr = x[c * BL:(c + 1) * BL, fi]          # [8, 256, 512]
            m[name] = np.ascontiguousarray(
                arr.transpose(2, 1, 0).reshape(DC, 128, R)).astype(f32)
        in_maps.append(m)
    return in_maps


def kernel(**inputs):
    from concourse.bass_utils import run_bass_kernel_spmd

    if "nc" not in _CACHE:
        _CACHE["nc"] = _build()
    nc = _CACHE["nc"]

    in_maps = _prep_inputs(**inputs)
    res = run_bass_kernel_spmd(
        nc, in_maps, core_ids=list(range(NCORES)),
        trace=bool(int(os.environ.get("LG_TRACE", "0"))))
    _CACHE["last_result"] = res

    outs = []
    for c in range(NCORES):
        hT = res.results[c]["hT_out"]                 # [4, 128, 256, 8]
        outs.append(np.ascontiguousarray(
            hT.transpose(3, 2, 0, 1).reshape(BL, T, H)))
    return np.concatenate(outs, axis=0).astype(np.float32)
